# revision 44
# baseline (speedup 1.0000x reference)
"""AttentionBlock (GroupNorm + degenerate head-axis attention + proj + residual)
on 8 Trainium2 NeuronCores, data-parallel over batch (2 batches per core).

Reference math (B=16, C=256, H=W=64, NH=4, dh=64, N=HW=4096, G=8 groups):
  xn   = GroupNorm(8, C)(x) * norm_w + norm_b
  qkv  = qkv_w @ xn + qkv_b            (1x1 convs == channel GEMMs)
  q,k,v: [NH, dh, N]; attn[p,i,j] = softmax_j( (1/8) sum_n q[i,p,n] k[j,p,n] )
  out[(p,i), n] = sum_j attn[p,i,j] v[(p,j), n]
  y    = proj_w @ out + proj_b + x

Kernel strategy (per core):
  - channels on partitions (2 chunks of 128), spatial N on the free dim
  - GroupNorm folded into the qkv GEMM: W_eff = W * scale(c) on the
    contraction channel; the bias side (W @ shift + b) enters the attention
    Gram as exact rank-1 fp32 corrections
  - q/k are computed TRANSPOSED ([n, channel], channels permuted p-major:
    pm = p*4 + head) so the 64 per-position 4x4 Gram matrices land on the
    diagonal 4-blocks of two [128, 128] blocks, accumulated across 32
    spatial chunks in PSUM
  - softmax over 4-blocks via a mask-bias tile (off-block -> -1e30 -> exp 0)
  - attention out and proj are fused: y = (EN.T @ PT).T @ v so the softmaxed
    Gram never needs a transpose
  - PRECISION="f32r": float32r matmuls (~1e-4 matmul rel err, y err ~2e-4)
    PRECISION="f16":  float16 matmuls (y err ~1.5e-3), faster weight loads
"""
import numpy as np

import concourse.bacc as bacc
import concourse.mybir as mybir
import concourse.tile as tile
from concourse.bass_utils import run_bass_kernel_spmd

F32 = mybir.dt.float32
F32R = mybir.dt.float32r
F16 = mybir.dt.float16

NCORES = 8
B, C, H, W = 16, 256, 64, 64
N = H * W                    # 4096
NB = B // NCORES             # batches per core = 2
NH, DH, G = 4, 64, 8
EPS = 1e-5
P = 128
NCH = C // P                 # channel chunks = 2
NT128 = N // 128             # 32
NT512 = N // 512             # 8
MASK_NEG = -1.0e30

PRECISION = "f16"            # "f32r" | "f16"

# p-major channel permutation: pm index j = p*4 + h  <->  orig channel h*64 + p
_PM = np.arange(C)
ORIG_OF_PM = (_PM % NH) * DH + _PM // NH   # orig channel for p-major index

_DEBUG_ACCUM_Y = False


def _build(replicate=1, loop=1, prec=None):
    """loop>1 wraps the computation in a hardware For_i repeating it
    (identical result every iteration) — used only for wall-clock timing."""
    prec = PRECISION if prec is None else prec
    CD = F32R if prec == "f32r" else F16          # compute dtype for big GEMMs
    f32r_mode = CD is F32R

    nc = bacc.Bacc()
    x_d = nc.declare_dram_parameter("x", [NB, C, N], CD, isOutput=False)
    wqk_d = nc.declare_dram_parameter("wqk", [C, 512], F32R, isOutput=False)
    wv_d = nc.declare_dram_parameter("wv", [C, C], F32R, isOutput=False)
    pt_d = nc.declare_dram_parameter("pt", [C, C], CD, isOutput=False)
    bqk_d = nc.declare_dram_parameter("bqk", [1, 512], F32, isOutput=False)
    bv_d = nc.declare_dram_parameter("bv", [P, NCH], F32, isOutput=False)
    pb_d = nc.declare_dram_parameter("pb", [P, NCH], F32, isOutput=False)
    nw_d = nc.declare_dram_parameter("nw", [P, NCH], F32, isOutput=False)
    nb_d = nc.declare_dram_parameter("nb", [P, NCH], F32, isOutput=False)
    ind_d = nc.declare_dram_parameter("ind", [P, NCH, G], F32, isOutput=False)
    bc_d = nc.declare_dram_parameter("bc", [G, NCH, P], F32, isOutput=False)
    mask_d = nc.declare_dram_parameter("mask", [P, P], F32, isOutput=False)
    YD = F32 if prec == "f32r" else F16
    y_d = nc.declare_dram_parameter("y", [NB, C, N], YD, isOutput=True)

    AOT = mybir.AluOpType
    AFT = mybir.ActivationFunctionType

    def f32view(ap):
        # DVE/ACT-readable view of a float32r AP
        return ap.bitcast(F32) if ap.dtype == F32R else ap

    with tile.TileContext(nc) as tc:
        with (
            tc.tile_pool(name="wpool", bufs=1) as wpool,       # weights/constants
            tc.tile_pool(name="xr", bufs=2) as xr_pool,        # x, per batch
            tc.tile_pool(name="vp", bufs=1) as v_pool,
            tc.tile_pool(name="qkt", bufs=3) as qkt_pool,
            tc.tile_pool(name="per_b", bufs=2) as pb_pool,     # per-batch smalls
            tc.tile_pool(name="ypool", bufs=3) as y_pool,
            tc.tile_pool(name="sm", bufs=2) as sm_pool,        # softmax temps
            tc.tile_pool(name="psb", bufs=3, space="PSUM") as ps_big,
            tc.tile_pool(name="psv", bufs=2, space="PSUM") as ps_vproj,
            tc.tile_pool(name="psg", bufs=2, space="PSUM") as ps_g,
            tc.tile_pool(name="pss", bufs=1, space="PSUM") as ps_small,
        ):
            # ---- load constants ----
            wqk_t = wpool.tile([P, NCH, 512], F32R)
            nc.sync.dma_start(wqk_t[:], wqk_d.rearrange("(m p) o -> p m o", p=P))
            wv_t = wpool.tile([P, NCH, C], F32R)
            nc.sync.dma_start(wv_t[:], wv_d.rearrange("(m p) o -> p m o", p=P))
            pt_t = wpool.tile([P, NCH, C], CD)
            nc.sync.dma_start(pt_t[:], pt_d.rearrange("(m p) o -> p m o", p=P))
            bqk_t = wpool.tile([1, 512], F32)
            nc.sync.dma_start(bqk_t[:], bqk_d[:])
            bv_t = wpool.tile([P, NCH], F32)
            nc.sync.dma_start(bv_t[:], bv_d[:])
            pb_t = wpool.tile([P, NCH], F32)
            nc.sync.dma_start(pb_t[:], pb_d[:])
            nw_t = wpool.tile([P, NCH], F32)
            nc.sync.dma_start(nw_t[:], nw_d[:])
            nb_t = wpool.tile([P, NCH], F32)
            nc.sync.dma_start(nb_t[:], nb_d[:])
            ind_t = wpool.tile([P, NCH, G], F32)
            nc.sync.dma_start(ind_t[:], ind_d[:])
            bc_t = wpool.tile([G, NCH, P], F32)
            nc.sync.dma_start(bc_t[:], bc_d[:])
            mask_t = wpool.tile([P, P], F32)
            nc.sync.dma_start(mask_t[:], mask_d[:])
            eps_t = wpool.tile([G, 1], F32)
            nc.vector.memset(eps_t[:], EPS)

            import contextlib
            loop_ctx = tc.For_i(0, loop, 1) if loop > 1 else contextlib.nullcontext()
            with loop_ctx:
              for b in [b for _ in range(replicate) for b in range(NB)]:
                # ================= load x + groupnorm stats =================
                x_r = xr_pool.tile([P, NCH, N], CD, tag="xr")
                stat2 = pb_pool.tile([P, NCH, 2], F32, tag="stat2")
                for m in range(NCH):
                    # half-granular DMA across both HWDGE rings
                    for q in range(2):
                        eng = nc.sync if (m + q) % 2 == 0 else nc.scalar
                        eng.dma_start(x_r[:, m, q * 2048:(q + 1) * 2048],
                                      x_d[b, m * P:(m + 1) * P, q * 2048:(q + 1) * 2048])
                    # per-channel mean/var over N
                    stats = pb_pool.tile([P, 8, 6], F32, tag="stats")
                    for j in range(8):
                        nc.vector.bn_stats(stats[:, j, :],
                                           f32view(x_r[:, m, j * 512:(j + 1) * 512]))
                    mv = pb_pool.tile([P, 2], F32, tag="mv")
                    nc.vector.bn_aggr(mv[:], stats[:])
                    # stat2 = (mean, var + mean^2)
                    nc.vector.tensor_copy(stat2[:, m, 0:1], mv[:, 0:1])
                    sq = pb_pool.tile([P, 1], F32, tag="sq")
                    nc.vector.tensor_mul(sq[:], mv[:, 0:1], mv[:, 0:1])
                    nc.vector.tensor_add(stat2[:, m, 1:2], mv[:, 1:2], sq[:])

                # group stats: S_g[g, :] = (mean_g, E[x^2]_g)   (IND holds 1/32)
                sg_ps = ps_small.tile([G, 2], F32, tag="small")
                for m in range(NCH):
                    nc.tensor.matmul(sg_ps[:], ind_t[:, m, :], stat2[:, m, :],
                                     start=(m == 0), stop=(m == NCH - 1))
                gs = pb_pool.tile([G, 2], F32, tag="gs")
                nc.vector.tensor_copy(gs[:], sg_ps[:])
                # var_g = E[x^2] - mean^2 ; rstd = 1/sqrt(var+eps)
                gsq = pb_pool.tile([G, 1], F32, tag="gsq")
                nc.vector.tensor_mul(gsq[:], gs[:, 0:1], gs[:, 0:1])
                gvar = pb_pool.tile([G, 1], F32, tag="gvar")
                nc.vector.tensor_tensor(gvar[:], gs[:, 1:2], gsq[:], AOT.subtract)
                grt = pb_pool.tile([G, 1], F32, tag="grt")
                nc.scalar.activation(grt[:], gvar[:], AFT.Sqrt, bias=eps_t[:])
                gstats = pb_pool.tile([G, 2], F32, tag="gstats")
                nc.vector.tensor_copy(gstats[:, 0:1], gs[:, 0:1])
                nc.vector.reciprocal(gstats[:, 1:2], grt[:])

                # per-channel scale/shift; effective weights; fold vectors
                wqk_eff = pb_pool.tile([P, NCH, 512], CD, tag="wqk_eff")
                wv_eff = pb_pool.tile([P, NCH, C], CD, tag="wv_eff")
                # duplicated to width 2: fp32r matmuls need even free dims
                shiftw = pb_pool.tile([P, NCH, 2], F32R, tag="shiftw")
                sxs2 = pb_pool.tile([P, NCH, 2], F32R, tag="sxs2")
                for m in range(NCH):
                    bc_ps = ps_small.tile([P, 2], F32, tag="small")
                    nc.tensor.matmul(bc_ps[:], bc_t[:, m, :], gstats[:], start=True, stop=True)
                    scale_m = pb_pool.tile([P, 1], F32, tag="scale_m")
                    nc.vector.tensor_mul(scale_m[:], bc_ps[:, 1:2], nw_t[:, m:m + 1])
                    tmp_m = pb_pool.tile([P, 1], F32, tag="tmp_m")
                    nc.vector.tensor_mul(tmp_m[:], bc_ps[:, 0:1], scale_m[:])
                    # shift = norm_b - mean*scale
                    nc.vector.tensor_tensor(shiftw[:, m, 0:1],
                                            nb_t[:, m:m + 1], tmp_m[:], AOT.subtract)
                    nc.vector.tensor_tensor(shiftw[:, m, 1:2],
                                            nb_t[:, m:m + 1], tmp_m[:], AOT.subtract)
                    # sxs = scale * sum_n x = scale * N * mean
                    sxs_m = pb_pool.tile([P, 1], F32, tag="sxs_m")
                    nc.vector.tensor_mul(sxs_m[:], stat2[:, m, 0:1], scale_m[:])
                    nc.vector.tensor_scalar_mul(sxs2[:, m, 0:1], sxs_m[:], float(N))
                    nc.vector.tensor_scalar_mul(sxs2[:, m, 1:2], sxs_m[:], float(N))
                    # effective weights (scale folded on the contraction dim)
                    nc.vector.tensor_scalar_mul(wqk_eff[:, m, :], f32view(wqk_t[:, m, :]), scale_m[:])
                    nc.vector.tensor_scalar_mul(wv_eff[:, m, :], f32view(wv_t[:, m, :]), scale_m[:])

                # qk effective bias row: beff = W.T @ shift + bqk   [1, 512]
                bq_ps = ps_small.tile([2, 512], F32, tag="small")
                for m in range(NCH):
                    nc.tensor.matmul(bq_ps[:], shiftw[:, m, :], wqk_t[:, m, :],
                                     start=(m == 0), stop=(m == NCH - 1))
                bq_sb = pb_pool.tile([1, 512], F32, tag="bq_sb")
                nc.vector.tensor_tensor(bq_sb[:], bq_ps[0:1, :], bqk_t[:], AOT.add)

                # wsx = W_eff.T @ sum_n x = W.T @ (scale * sx)  -> [1, 512]
                wsx_ps = ps_small.tile([2, 512], F32, tag="small")
                for m in range(NCH):
                    nc.tensor.matmul(wsx_ps[:], sxs2[:, m, :], wqk_t[:, m, :],
                                     start=(m == 0), stop=(m == NCH - 1))
                wsx_sb = pb_pool.tile([1, 512], F32, tag="wsx_sb")
                nc.vector.tensor_copy(wsx_sb[:], wsx_ps[0:1, :])
                # wcomb = Wk_eff sx + N * beff_k
                nbk = pb_pool.tile([1, C], F32, tag="nbk")
                nc.vector.tensor_scalar_mul(nbk[:], bq_sb[0:1, C:2 * C], float(N))
                wcomb = pb_pool.tile([1, C], F32, tag="wcomb")
                nc.vector.tensor_tensor(wcomb[:], wsx_sb[0:1, C:2 * C], nbk[:], AOT.add)

                # v bias: bveff[:, oc] = Wv.T @ shift + bv
                bveff = pb_pool.tile([P, NCH], F32, tag="bveff")
                for oc in range(NCH):
                    bv_ps = ps_small.tile([P, 2], F32, tag="small")
                    for m in range(NCH):
                        nc.tensor.matmul(bv_ps[:], wv_t[:, m, oc * P:(oc + 1) * P],
                                         shiftw[:, m, :],
                                         start=(m == 0), stop=(m == NCH - 1))
                    nc.vector.tensor_tensor(bveff[:, oc:oc + 1], bv_ps[:, 0:1],
                                            bv_t[:, oc:oc + 1], AOT.add)

                # ================= qkT sweep + Gram accumulation =================
                # qkT tiles hold UNBIASED q0/k0 (channels p-major, q pre-scaled
                # by 1/8 on the host); biases enter as rank-1 fp32 terms:
                #   G = q0 k0' + beffq (Wk_eff sx + N beffk)' + (Wq_eff sx) beffk'
                g_ps = [ps_g.tile([P, C], F32, tag="g", name=f"g_ps_{b}_{m}") for m in range(NCH)]
                for t in range(NT128):
                    qk_ps = ps_big.tile([P, 512], F32, tag="big")
                    for m in range(NCH):
                        nc.tensor.matmul(qk_ps[:], x_r[:, m, t * P:(t + 1) * P],
                                         wqk_eff[:, m, :],
                                         start=(m == 0), stop=(m == NCH - 1))
                    qkt = qkt_pool.tile([P, 512], CD, tag="qkt")
                    if t % 2 == 0:
                        nc.vector.tensor_copy(qkt[:], qk_ps[:])
                    else:
                        nc.scalar.copy(qkt[:], qk_ps[:])
                    if f32r_mode:
                        # f32r needs moving free >= 256: compute full k columns
                        for m in range(NCH):
                            nc.tensor.matmul(g_ps[m][:], qkt[:, m * P:(m + 1) * P],
                                             qkt[:, 256:512],
                                             start=(t == 0), stop=False)
                    else:
                        # f16 full-rate at N=128: only the relevant k chunk
                        for m in range(NCH):
                            nc.tensor.matmul(g_ps[m][:, m * P:(m + 1) * P],
                                             qkt[:, m * P:(m + 1) * P],
                                             qkt[:, 256 + m * P:256 + (m + 1) * P],
                                             start=(t == 0), stop=False)
                # rank-1 bias corrections (plain fp32 — exact)
                for m in range(NCH):
                    if f32r_mode:
                        nc.tensor.matmul(g_ps[m][:], bq_sb[0:1, m * P:(m + 1) * P],
                                         wcomb[:], start=False, stop=False)
                        nc.tensor.matmul(g_ps[m][:], wsx_sb[0:1, m * P:(m + 1) * P],
                                         bq_sb[0:1, C:2 * C], start=False, stop=True)
                    else:
                        nc.tensor.matmul(g_ps[m][:, m * P:(m + 1) * P],
                                         bq_sb[0:1, m * P:(m + 1) * P],
                                         wcomb[0:1, m * P:(m + 1) * P],
                                         start=False, stop=False)
                        nc.tensor.matmul(g_ps[m][:, m * P:(m + 1) * P],
                                         wsx_sb[0:1, m * P:(m + 1) * P],
                                         bq_sb[0:1, C + m * P:C + (m + 1) * P],
                                         start=False, stop=True)

                # ================= v GEMM / softmax / proj =================
                v_t = v_pool.tile([P, NCH, N], CD, tag="v")

                def emit_v(nt, oc):
                    v_ps = ps_vproj.tile([P, 512], F32, tag="vproj", name="v_ps")
                    for m in range(NCH):
                        nc.tensor.matmul(v_ps[:], wv_eff[:, m, oc * P:(oc + 1) * P],
                                         x_r[:, m, nt * 512:(nt + 1) * 512],
                                         start=(m == 0), stop=(m == NCH - 1))
                    nc.scalar.activation(v_t[:, oc, nt * 512:(nt + 1) * 512], v_ps[:],
                                         AFT.Identity, bias=bveff[:, oc:oc + 1])

                def emit_proj(nt, oc):
                    y_ps = ps_vproj.tile([P, 512], F32, tag="vproj", name="y_ps")
                    for m in range(NCH):
                        nc.tensor.matmul(y_ps[:], qt_t[:, m, oc * P:(oc + 1) * P],
                                         v_t[:, m, nt * 512:(nt + 1) * 512],
                                         start=(m == 0), stop=(m == NCH - 1))
                    y_sb = y_pool.tile([P, 512], YD, tag="y", name="y_sb")
                    # y = (psum + proj_b) + x
                    nc.vector.scalar_tensor_tensor(
                        y_sb[:], y_ps[:], pb_t[:, oc:oc + 1],
                        f32view(x_r[:, oc, nt * 512:(nt + 1) * 512]),
                        AOT.add, AOT.add)
                    if _DEBUG_ACCUM_Y:
                        nc.gpsimd.dma_start(
                            y_d[b, oc * P:(oc + 1) * P, nt * 512:(nt + 1) * 512],
                            y_sb[:], accum_op=AOT.add)
                    else:
                        # ~20% of stores on the sync ring (which also carries x
                        # loads) to balance the two HWDGE rings
                        seng = nc.sync if (nt * NCH + oc) % 5 == 0 else nc.scalar
                        seng.dma_start(
                            y_d[b, oc * P:(oc + 1) * P, nt * 512:(nt + 1) * 512],
                            y_sb[:])

                V_PREFIX = 5
                for nt in range(V_PREFIX):
                    for oc in range(NCH):
                        emit_v(nt, oc)

                # softmax + QT (DVE/ACT chain; PE covered by the v prefix)
                qt_t = pb_pool.tile([P, NCH, C], CD, tag="qt")
                for m in range(NCH):
                    grel = g_ps[m][:, m * P:(m + 1) * P]
                    s_t = sm_pool.tile([P, P], F32, tag="s")
                    nc.vector.tensor_tensor(s_t[:], grel, mask_t[:], AOT.add)
                    mx = sm_pool.tile([P, 1], F32, tag="mx")
                    nc.vector.reduce_max(mx[:], s_t[:], axis=mybir.AxisListType.X)
                    mxn = sm_pool.tile([P, 1], F32, tag="mxn")
                    nc.vector.tensor_scalar_mul(mxn[:], mx[:], -1.0)
                    e_t = sm_pool.tile([P, P], F32, tag="e")
                    esum = sm_pool.tile([P, 1], F32, tag="esum")
                    nc.scalar.activation(e_t[:], s_t[:], AFT.Exp, bias=mxn[:], accum_out=esum[:])
                    erec = sm_pool.tile([P, 1], F32, tag="erec")
                    nc.vector.reciprocal(erec[:], esum[:])
                    en_t = sm_pool.tile([P, P], CD, tag="en")
                    nc.vector.tensor_scalar_mul(en_t[:], e_t[:], erec[:])
                    # QT_m = EN_m.T @ PT_m   (softmaxed Gram enters transposed)
                    qt_ps = ps_small.tile([P, C], F32, tag="small")
                    nc.tensor.matmul(qt_ps[:], en_t[:], pt_t[:, m, :], start=True, stop=True)
                    nc.vector.tensor_copy(qt_t[:, m, :], qt_ps[:])

                # interleave remaining v with proj (balances PE vs DVE/ACT drains)
                for nt in range(V_PREFIX, NT512):
                    for oc in range(NCH):
                        emit_v(nt, oc)
                    pnt = nt - V_PREFIX
                    for oc in range(NCH):
                        emit_proj(pnt, oc)
                for nt in range(NT512 - V_PREFIX, NT512):
                    for oc in range(NCH):
                        emit_proj(nt, oc)

    if not nc.is_finalized():
        nc.finalize()
    return nc


_NC_CACHE = {}


def _get_nc(replicate=1, loop=1, prec=None):
    prec = PRECISION if prec is None else prec
    key = (replicate, loop, prec)
    if key not in _NC_CACHE:
        _NC_CACHE[key] = _build(replicate, loop, prec)
    return _NC_CACHE[key]


def _host_inputs(x, norm_w, norm_b, qkv_w, qkv_b, proj_w, proj_b, prec):
    """Host-side weight preprocessing -> per-core common input dict."""
    f = np.float32
    cd = np.float32 if prec == "f32r" else np.float16
    norm_w, norm_b = np.asarray(norm_w, f), np.asarray(norm_b, f)
    qkv_w, qkv_b = np.asarray(qkv_w, f), np.asarray(qkv_b, f)
    proj_w, proj_b = np.asarray(proj_w, f), np.asarray(proj_b, f)

    perm = ORIG_OF_PM
    wq = qkv_w[0:C][perm] / 8.0          # fold attention scale dh^-0.5 = 1/8
    wk = qkv_w[C:2 * C][perm]
    wv = qkv_w[2 * C:3 * C][perm]
    bq = qkv_b[0:C][perm] / 8.0
    bk = qkv_b[C:2 * C][perm]
    bv = qkv_b[2 * C:3 * C][perm]

    wqk = np.concatenate([wq.T, wk.T], axis=1).astype(f)      # [C, 512]
    bqk = np.concatenate([bq, bk])[None, :].astype(f)         # [1, 512]
    wv_c = np.ascontiguousarray(wv.T).astype(f)               # [C, C] (c_in, o_pm)
    pt = np.ascontiguousarray(proj_w[:, perm].T).astype(cd)   # [C(pm), C(orig o)]

    ch = np.arange(C)
    ind = np.zeros((P, NCH, G), f)
    bc = np.zeros((G, NCH, P), f)
    for m in range(NCH):
        grp = (ch[m * P:(m + 1) * P]) // (C // G)
        for c0 in range(P):
            ind[c0, m, grp[c0]] = 1.0 / (C // G)
            bc[grp[c0], m, c0] = 1.0
    a = np.arange(P)
    mask = np.where((a[:, None] // NH) == (a[None, :] // NH), 0.0, MASK_NEG).astype(f)

    def chunk2(v_):  # [C] -> [P, NCH]
        return np.stack([v_[m * P:(m + 1) * P] for m in range(NCH)], axis=1).astype(f)

    return {
        "wqk": wqk, "wv": wv_c, "pt": pt, "bqk": bqk,
        "bv": chunk2(bv), "pb": chunk2(proj_b),
        "nw": chunk2(norm_w), "nb": chunk2(norm_b),
        "ind": ind, "bc": bc, "mask": mask,
    }


def make_in_maps(x, norm_w, norm_b, qkv_w, qkv_b, proj_w, proj_b, prec=None):
    prec = PRECISION if prec is None else prec
    cd = np.float32 if prec == "f32r" else np.float16
    common = _host_inputs(x, norm_w, norm_b, qkv_w, qkv_b, proj_w, proj_b, prec)
    xr = np.ascontiguousarray(
        np.asarray(x, dtype=np.float32).reshape(B, C, N).astype(cd))
    in_maps = []
    for c in range(NCORES):
        m = dict(common)
        m["x"] = xr[c * NB:(c + 1) * NB]
        in_maps.append(m)
    return in_maps


def kernel(x, norm_w, norm_b, qkv_w, qkv_b, proj_w, proj_b):
    nc = _get_nc()
    in_maps = make_in_maps(x, norm_w, norm_b, qkv_w, qkv_b, proj_w, proj_b)
    res = run_bass_kernel_spmd(nc, in_maps, core_ids=list(range(NCORES)))
    y = np.concatenate([res.results[c]["y"] for c in range(NCORES)], axis=0)
    return y.reshape(B, C, H, W).astype(np.float32)


# revision 45
# speedup vs baseline: 1.0160x; 1.0160x over previous
"""AttentionBlock (GroupNorm + degenerate head-axis attention + proj + residual)
on 8 Trainium2 NeuronCores, data-parallel over batch (2 batches per core).

Reference math (B=16, C=256, H=W=64, NH=4, dh=64, N=HW=4096, G=8 groups):
  xn   = GroupNorm(8, C)(x) * norm_w + norm_b
  qkv  = qkv_w @ xn + qkv_b            (1x1 convs == channel GEMMs)
  q,k,v: [NH, dh, N]; attn[p,i,j] = softmax_j( (1/8) sum_n q[i,p,n] k[j,p,n] )
  out[(p,i), n] = sum_j attn[p,i,j] v[(p,j), n]
  y    = proj_w @ out + proj_b + x

Kernel strategy (per core):
  - channels on partitions (2 chunks of 128), spatial N on the free dim
  - GroupNorm folded into the qkv GEMM: W_eff = W * scale(c) on the
    contraction channel; the bias side (W @ shift + b) enters the attention
    Gram as exact rank-1 fp32 corrections
  - q/k are computed TRANSPOSED ([n, channel], channels permuted p-major:
    pm = p*4 + head) so the 64 per-position 4x4 Gram matrices land on the
    diagonal 4-blocks of two [128, 128] blocks, accumulated across 32
    spatial chunks in PSUM
  - softmax over 4-blocks via a mask-bias tile (off-block -> -1e30 -> exp 0)
  - attention out and proj are fused: y = (EN.T @ PT).T @ v so the softmaxed
    Gram never needs a transpose
  - PRECISION="f32r": float32r matmuls (~1e-4 matmul rel err, y err ~2e-4)
    PRECISION="f16":  float16 matmuls (y err ~1.5e-3), faster weight loads
"""
import numpy as np

import concourse.bacc as bacc
import concourse.mybir as mybir
import concourse.tile as tile
from concourse.bass_utils import run_bass_kernel_spmd

F32 = mybir.dt.float32
F32R = mybir.dt.float32r
F16 = mybir.dt.float16

NCORES = 8
B, C, H, W = 16, 256, 64, 64
N = H * W                    # 4096
NB = B // NCORES             # batches per core = 2
NH, DH, G = 4, 64, 8
EPS = 1e-5
P = 128
NCH = C // P                 # channel chunks = 2
NT128 = N // 128             # 32
NT512 = N // 512             # 8
MASK_NEG = -1.0e30

PRECISION = "f16"            # "f32r" | "f16"

# p-major channel permutation: pm index j = p*4 + h  <->  orig channel h*64 + p
_PM = np.arange(C)
ORIG_OF_PM = (_PM % NH) * DH + _PM // NH   # orig channel for p-major index

_DEBUG_ACCUM_Y = False


def _build(replicate=1, loop=1, prec=None):
    """loop>1 wraps the computation in a hardware For_i repeating it
    (identical result every iteration) — used only for wall-clock timing."""
    prec = PRECISION if prec is None else prec
    CD = F32R if prec == "f32r" else F16          # compute dtype for big GEMMs
    f32r_mode = CD is F32R

    nc = bacc.Bacc()
    x_d = nc.declare_dram_parameter("x", [NB, C, N], CD, isOutput=False)
    wqk_d = nc.declare_dram_parameter("wqk", [C, 512], F32R, isOutput=False)
    wv_d = nc.declare_dram_parameter("wv", [C, C], F32R, isOutput=False)
    pt_d = nc.declare_dram_parameter("pt", [C, C], CD, isOutput=False)
    bqk_d = nc.declare_dram_parameter("bqk", [1, 512], F32, isOutput=False)
    bv_d = nc.declare_dram_parameter("bv", [P, NCH], F32, isOutput=False)
    pb_d = nc.declare_dram_parameter("pb", [P, NCH], F32, isOutput=False)
    nw_d = nc.declare_dram_parameter("nw", [P, NCH], F32, isOutput=False)
    nb_d = nc.declare_dram_parameter("nb", [P, NCH], F32, isOutput=False)
    ind_d = nc.declare_dram_parameter("ind", [P, NCH, G], F32, isOutput=False)
    bc_d = nc.declare_dram_parameter("bc", [G, NCH, P], F32, isOutput=False)
    mask_d = nc.declare_dram_parameter("mask", [P, P], F32, isOutput=False)
    YD = F32 if prec == "f32r" else F16
    y_d = nc.declare_dram_parameter("y", [NB, C, N], YD, isOutput=True)

    AOT = mybir.AluOpType
    AFT = mybir.ActivationFunctionType

    def f32view(ap):
        # DVE/ACT-readable view of a float32r AP
        return ap.bitcast(F32) if ap.dtype == F32R else ap

    with tile.TileContext(nc) as tc:
        with (
            tc.tile_pool(name="wpool", bufs=1) as wpool,       # weights/constants
            tc.tile_pool(name="xr", bufs=2) as xr_pool,        # x, per batch
            tc.tile_pool(name="vp", bufs=1) as v_pool,
            tc.tile_pool(name="qkt", bufs=6) as qkt_pool,
            tc.tile_pool(name="per_b", bufs=2) as pb_pool,     # per-batch smalls
            tc.tile_pool(name="ypool", bufs=5) as y_pool,
            tc.tile_pool(name="sm", bufs=3) as sm_pool,        # softmax temps
            tc.tile_pool(name="psb", bufs=3, space="PSUM") as ps_big,
            tc.tile_pool(name="psv", bufs=2, space="PSUM") as ps_vproj,
            tc.tile_pool(name="psg", bufs=2, space="PSUM") as ps_g,
            tc.tile_pool(name="pss", bufs=1, space="PSUM") as ps_small,
        ):
            # ---- load constants ----
            wqk_t = wpool.tile([P, NCH, 512], F32R)
            nc.sync.dma_start(wqk_t[:], wqk_d.rearrange("(m p) o -> p m o", p=P))
            wv_t = wpool.tile([P, NCH, C], F32R)
            nc.sync.dma_start(wv_t[:], wv_d.rearrange("(m p) o -> p m o", p=P))
            pt_t = wpool.tile([P, NCH, C], CD)
            nc.sync.dma_start(pt_t[:], pt_d.rearrange("(m p) o -> p m o", p=P))
            bqk_t = wpool.tile([1, 512], F32)
            nc.sync.dma_start(bqk_t[:], bqk_d[:])
            bv_t = wpool.tile([P, NCH], F32)
            nc.sync.dma_start(bv_t[:], bv_d[:])
            pb_t = wpool.tile([P, NCH], F32)
            nc.sync.dma_start(pb_t[:], pb_d[:])
            nw_t = wpool.tile([P, NCH], F32)
            nc.sync.dma_start(nw_t[:], nw_d[:])
            nb_t = wpool.tile([P, NCH], F32)
            nc.sync.dma_start(nb_t[:], nb_d[:])
            ind_t = wpool.tile([P, NCH, G], F32)
            nc.sync.dma_start(ind_t[:], ind_d[:])
            bc_t = wpool.tile([G, NCH, P], F32)
            nc.sync.dma_start(bc_t[:], bc_d[:])
            mask_t = wpool.tile([P, P], F32)
            nc.sync.dma_start(mask_t[:], mask_d[:])
            eps_t = wpool.tile([G, 1], F32)
            nc.vector.memset(eps_t[:], EPS)

            import contextlib
            loop_ctx = tc.For_i(0, loop, 1) if loop > 1 else contextlib.nullcontext()
            with loop_ctx:
              for b in [b for _ in range(replicate) for b in range(NB)]:
                # ================= load x + groupnorm stats =================
                x_r = xr_pool.tile([P, NCH, N], CD, tag="xr")
                stat2 = pb_pool.tile([P, NCH, 2], F32, tag="stat2")
                for m in range(NCH):
                    # half-granular DMA across both HWDGE rings
                    for q in range(2):
                        eng = nc.sync if (m + q) % 2 == 0 else nc.scalar
                        eng.dma_start(x_r[:, m, q * 2048:(q + 1) * 2048],
                                      x_d[b, m * P:(m + 1) * P, q * 2048:(q + 1) * 2048])
                    # per-channel mean/var over N
                    stats = pb_pool.tile([P, 8, 6], F32, tag="stats")
                    for j in range(8):
                        nc.vector.bn_stats(stats[:, j, :],
                                           f32view(x_r[:, m, j * 512:(j + 1) * 512]))
                    mv = pb_pool.tile([P, 2], F32, tag="mv")
                    nc.vector.bn_aggr(mv[:], stats[:])
                    # stat2 = (mean, var + mean^2)
                    nc.vector.tensor_copy(stat2[:, m, 0:1], mv[:, 0:1])
                    sq = pb_pool.tile([P, 1], F32, tag="sq")
                    nc.vector.tensor_mul(sq[:], mv[:, 0:1], mv[:, 0:1])
                    nc.vector.tensor_add(stat2[:, m, 1:2], mv[:, 1:2], sq[:])

                # group stats: S_g[g, :] = (mean_g, E[x^2]_g)   (IND holds 1/32)
                sg_ps = ps_small.tile([G, 2], F32, tag="small")
                for m in range(NCH):
                    nc.tensor.matmul(sg_ps[:], ind_t[:, m, :], stat2[:, m, :],
                                     start=(m == 0), stop=(m == NCH - 1))
                gs = pb_pool.tile([G, 2], F32, tag="gs")
                nc.vector.tensor_copy(gs[:], sg_ps[:])
                # var_g = E[x^2] - mean^2 ; rstd = 1/sqrt(var+eps)
                gsq = pb_pool.tile([G, 1], F32, tag="gsq")
                nc.vector.tensor_mul(gsq[:], gs[:, 0:1], gs[:, 0:1])
                gvar = pb_pool.tile([G, 1], F32, tag="gvar")
                nc.vector.tensor_tensor(gvar[:], gs[:, 1:2], gsq[:], AOT.subtract)
                grt = pb_pool.tile([G, 1], F32, tag="grt")
                nc.scalar.activation(grt[:], gvar[:], AFT.Sqrt, bias=eps_t[:])
                gstats = pb_pool.tile([G, 2], F32, tag="gstats")
                nc.vector.tensor_copy(gstats[:, 0:1], gs[:, 0:1])
                nc.vector.reciprocal(gstats[:, 1:2], grt[:])

                # per-channel scale/shift; effective weights; fold vectors
                wqk_eff = pb_pool.tile([P, NCH, 512], CD, tag="wqk_eff")
                wv_eff = pb_pool.tile([P, NCH, C], CD, tag="wv_eff")
                # duplicated to width 2: fp32r matmuls need even free dims
                shiftw = pb_pool.tile([P, NCH, 2], F32R, tag="shiftw")
                sxs2 = pb_pool.tile([P, NCH, 2], F32R, tag="sxs2")
                for m in range(NCH):
                    bc_ps = ps_small.tile([P, 2], F32, tag="small")
                    nc.tensor.matmul(bc_ps[:], bc_t[:, m, :], gstats[:], start=True, stop=True)
                    scale_m = pb_pool.tile([P, 1], F32, tag="scale_m")
                    nc.vector.tensor_mul(scale_m[:], bc_ps[:, 1:2], nw_t[:, m:m + 1])
                    tmp_m = pb_pool.tile([P, 1], F32, tag="tmp_m")
                    nc.vector.tensor_mul(tmp_m[:], bc_ps[:, 0:1], scale_m[:])
                    # shift = norm_b - mean*scale
                    nc.vector.tensor_tensor(shiftw[:, m, 0:1],
                                            nb_t[:, m:m + 1], tmp_m[:], AOT.subtract)
                    nc.vector.tensor_tensor(shiftw[:, m, 1:2],
                                            nb_t[:, m:m + 1], tmp_m[:], AOT.subtract)
                    # sxs = scale * sum_n x = scale * N * mean
                    sxs_m = pb_pool.tile([P, 1], F32, tag="sxs_m")
                    nc.vector.tensor_mul(sxs_m[:], stat2[:, m, 0:1], scale_m[:])
                    nc.vector.tensor_scalar_mul(sxs2[:, m, 0:1], sxs_m[:], float(N))
                    nc.vector.tensor_scalar_mul(sxs2[:, m, 1:2], sxs_m[:], float(N))
                    # effective weights (scale folded on the contraction dim)
                    nc.vector.tensor_scalar_mul(wqk_eff[:, m, :], f32view(wqk_t[:, m, :]), scale_m[:])
                    nc.vector.tensor_scalar_mul(wv_eff[:, m, :], f32view(wv_t[:, m, :]), scale_m[:])

                # qk effective bias row: beff = W.T @ shift + bqk   [1, 512]
                bq_ps = ps_small.tile([2, 512], F32, tag="small")
                for m in range(NCH):
                    nc.tensor.matmul(bq_ps[:], shiftw[:, m, :], wqk_t[:, m, :],
                                     start=(m == 0), stop=(m == NCH - 1))
                bq_sb = pb_pool.tile([1, 512], F32, tag="bq_sb")
                nc.vector.tensor_tensor(bq_sb[:], bq_ps[0:1, :], bqk_t[:], AOT.add)

                # wsx = W_eff.T @ sum_n x = W.T @ (scale * sx)  -> [1, 512]
                wsx_ps = ps_small.tile([2, 512], F32, tag="small")
                for m in range(NCH):
                    nc.tensor.matmul(wsx_ps[:], sxs2[:, m, :], wqk_t[:, m, :],
                                     start=(m == 0), stop=(m == NCH - 1))
                wsx_sb = pb_pool.tile([1, 512], F32, tag="wsx_sb")
                nc.vector.tensor_copy(wsx_sb[:], wsx_ps[0:1, :])
                # wcomb = Wk_eff sx + N * beff_k
                nbk = pb_pool.tile([1, C], F32, tag="nbk")
                nc.vector.tensor_scalar_mul(nbk[:], bq_sb[0:1, C:2 * C], float(N))
                wcomb = pb_pool.tile([1, C], F32, tag="wcomb")
                nc.vector.tensor_tensor(wcomb[:], wsx_sb[0:1, C:2 * C], nbk[:], AOT.add)

                # v bias: bveff[:, oc] = Wv.T @ shift + bv
                bveff = pb_pool.tile([P, NCH], F32, tag="bveff")
                for oc in range(NCH):
                    bv_ps = ps_small.tile([P, 2], F32, tag="small")
                    for m in range(NCH):
                        nc.tensor.matmul(bv_ps[:], wv_t[:, m, oc * P:(oc + 1) * P],
                                         shiftw[:, m, :],
                                         start=(m == 0), stop=(m == NCH - 1))
                    nc.vector.tensor_tensor(bveff[:, oc:oc + 1], bv_ps[:, 0:1],
                                            bv_t[:, oc:oc + 1], AOT.add)

                # ================= qkT sweep + Gram accumulation =================
                # qkT tiles hold UNBIASED q0/k0 (channels p-major, q pre-scaled
                # by 1/8 on the host); biases enter as rank-1 fp32 terms:
                #   G = q0 k0' + beffq (Wk_eff sx + N beffk)' + (Wq_eff sx) beffk'
                g_ps = [ps_g.tile([P, C], F32, tag="g", name=f"g_ps_{b}_{m}") for m in range(NCH)]
                for t in range(NT128):
                    qk_ps = ps_big.tile([P, 512], F32, tag="big")
                    for m in range(NCH):
                        nc.tensor.matmul(qk_ps[:], x_r[:, m, t * P:(t + 1) * P],
                                         wqk_eff[:, m, :],
                                         start=(m == 0), stop=(m == NCH - 1))
                    qkt = qkt_pool.tile([P, 512], CD, tag="qkt")
                    if t % 2 == 0:
                        nc.vector.tensor_copy(qkt[:], qk_ps[:])
                    else:
                        nc.scalar.copy(qkt[:], qk_ps[:])
                    if f32r_mode:
                        # f32r needs moving free >= 256: compute full k columns
                        for m in range(NCH):
                            nc.tensor.matmul(g_ps[m][:], qkt[:, m * P:(m + 1) * P],
                                             qkt[:, 256:512],
                                             start=(t == 0), stop=False)
                    else:
                        # f16 full-rate at N=128: only the relevant k chunk
                        for m in range(NCH):
                            nc.tensor.matmul(g_ps[m][:, m * P:(m + 1) * P],
                                             qkt[:, m * P:(m + 1) * P],
                                             qkt[:, 256 + m * P:256 + (m + 1) * P],
                                             start=(t == 0), stop=False)
                # rank-1 bias corrections (plain fp32 — exact)
                for m in range(NCH):
                    if f32r_mode:
                        nc.tensor.matmul(g_ps[m][:], bq_sb[0:1, m * P:(m + 1) * P],
                                         wcomb[:], start=False, stop=False)
                        nc.tensor.matmul(g_ps[m][:], wsx_sb[0:1, m * P:(m + 1) * P],
                                         bq_sb[0:1, C:2 * C], start=False, stop=True)
                    else:
                        nc.tensor.matmul(g_ps[m][:, m * P:(m + 1) * P],
                                         bq_sb[0:1, m * P:(m + 1) * P],
                                         wcomb[0:1, m * P:(m + 1) * P],
                                         start=False, stop=False)
                        nc.tensor.matmul(g_ps[m][:, m * P:(m + 1) * P],
                                         wsx_sb[0:1, m * P:(m + 1) * P],
                                         bq_sb[0:1, C + m * P:C + (m + 1) * P],
                                         start=False, stop=True)

                # ================= v GEMM / softmax / proj =================
                v_t = v_pool.tile([P, NCH, N], CD, tag="v")

                def emit_v(nt, oc):
                    v_ps = ps_vproj.tile([P, 512], F32, tag="vproj", name="v_ps")
                    for m in range(NCH):
                        nc.tensor.matmul(v_ps[:], wv_eff[:, m, oc * P:(oc + 1) * P],
                                         x_r[:, m, nt * 512:(nt + 1) * 512],
                                         start=(m == 0), stop=(m == NCH - 1))
                    nc.scalar.activation(v_t[:, oc, nt * 512:(nt + 1) * 512], v_ps[:],
                                         AFT.Identity, bias=bveff[:, oc:oc + 1])

                def emit_proj(nt, oc):
                    y_ps = ps_vproj.tile([P, 512], F32, tag="vproj", name="y_ps")
                    for m in range(NCH):
                        nc.tensor.matmul(y_ps[:], qt_t[:, m, oc * P:(oc + 1) * P],
                                         v_t[:, m, nt * 512:(nt + 1) * 512],
                                         start=(m == 0), stop=(m == NCH - 1))
                    y_sb = y_pool.tile([P, 512], YD, tag="y", name="y_sb")
                    # y = (psum + proj_b) + x
                    nc.vector.scalar_tensor_tensor(
                        y_sb[:], y_ps[:], pb_t[:, oc:oc + 1],
                        f32view(x_r[:, oc, nt * 512:(nt + 1) * 512]),
                        AOT.add, AOT.add)
                    if _DEBUG_ACCUM_Y:
                        nc.gpsimd.dma_start(
                            y_d[b, oc * P:(oc + 1) * P, nt * 512:(nt + 1) * 512],
                            y_sb[:], accum_op=AOT.add)
                    else:
                        # ~20% of stores on the sync ring (which also carries x
                        # loads) to balance the two HWDGE rings
                        seng = nc.sync if (nt * NCH + oc) % 5 == 0 else nc.scalar
                        seng.dma_start(
                            y_d[b, oc * P:(oc + 1) * P, nt * 512:(nt + 1) * 512],
                            y_sb[:])

                V_PREFIX = 5
                for nt in range(V_PREFIX):
                    for oc in range(NCH):
                        emit_v(nt, oc)

                # softmax + QT (DVE/ACT chain; PE covered by the v prefix)
                qt_t = pb_pool.tile([P, NCH, C], CD, tag="qt")
                for m in range(NCH):
                    grel = g_ps[m][:, m * P:(m + 1) * P]
                    s_t = sm_pool.tile([P, P], F32, tag="s")
                    nc.vector.tensor_tensor(s_t[:], grel, mask_t[:], AOT.add)
                    mx = sm_pool.tile([P, 1], F32, tag="mx")
                    nc.vector.reduce_max(mx[:], s_t[:], axis=mybir.AxisListType.X)
                    mxn = sm_pool.tile([P, 1], F32, tag="mxn")
                    nc.vector.tensor_scalar_mul(mxn[:], mx[:], -1.0)
                    e_t = sm_pool.tile([P, P], F32, tag="e")
                    esum = sm_pool.tile([P, 1], F32, tag="esum")
                    nc.scalar.activation(e_t[:], s_t[:], AFT.Exp, bias=mxn[:], accum_out=esum[:])
                    erec = sm_pool.tile([P, 1], F32, tag="erec")
                    nc.vector.reciprocal(erec[:], esum[:])
                    en_t = sm_pool.tile([P, P], CD, tag="en")
                    nc.vector.tensor_scalar_mul(en_t[:], e_t[:], erec[:])
                    # QT_m = EN_m.T @ PT_m   (softmaxed Gram enters transposed)
                    qt_ps = ps_small.tile([P, C], F32, tag="small")
                    nc.tensor.matmul(qt_ps[:], en_t[:], pt_t[:, m, :], start=True, stop=True)
                    nc.vector.tensor_copy(qt_t[:, m, :], qt_ps[:])

                # interleave remaining v with proj (balances PE vs DVE/ACT drains)
                for nt in range(V_PREFIX, NT512):
                    for oc in range(NCH):
                        emit_v(nt, oc)
                    pnt = nt - V_PREFIX
                    for oc in range(NCH):
                        emit_proj(pnt, oc)
                for nt in range(NT512 - V_PREFIX, NT512):
                    for oc in range(NCH):
                        emit_proj(nt, oc)

    if not nc.is_finalized():
        nc.finalize()
    return nc


_NC_CACHE = {}


def _get_nc(replicate=1, loop=1, prec=None):
    prec = PRECISION if prec is None else prec
    key = (replicate, loop, prec)
    if key not in _NC_CACHE:
        _NC_CACHE[key] = _build(replicate, loop, prec)
    return _NC_CACHE[key]


def _host_inputs(x, norm_w, norm_b, qkv_w, qkv_b, proj_w, proj_b, prec):
    """Host-side weight preprocessing -> per-core common input dict."""
    f = np.float32
    cd = np.float32 if prec == "f32r" else np.float16
    norm_w, norm_b = np.asarray(norm_w, f), np.asarray(norm_b, f)
    qkv_w, qkv_b = np.asarray(qkv_w, f), np.asarray(qkv_b, f)
    proj_w, proj_b = np.asarray(proj_w, f), np.asarray(proj_b, f)

    perm = ORIG_OF_PM
    wq = qkv_w[0:C][perm] / 8.0          # fold attention scale dh^-0.5 = 1/8
    wk = qkv_w[C:2 * C][perm]
    wv = qkv_w[2 * C:3 * C][perm]
    bq = qkv_b[0:C][perm] / 8.0
    bk = qkv_b[C:2 * C][perm]
    bv = qkv_b[2 * C:3 * C][perm]

    wqk = np.concatenate([wq.T, wk.T], axis=1).astype(f)      # [C, 512]
    bqk = np.concatenate([bq, bk])[None, :].astype(f)         # [1, 512]
    wv_c = np.ascontiguousarray(wv.T).astype(f)               # [C, C] (c_in, o_pm)
    pt = np.ascontiguousarray(proj_w[:, perm].T).astype(cd)   # [C(pm), C(orig o)]

    ch = np.arange(C)
    ind = np.zeros((P, NCH, G), f)
    bc = np.zeros((G, NCH, P), f)
    for m in range(NCH):
        grp = (ch[m * P:(m + 1) * P]) // (C // G)
        for c0 in range(P):
            ind[c0, m, grp[c0]] = 1.0 / (C // G)
            bc[grp[c0], m, c0] = 1.0
    a = np.arange(P)
    mask = np.where((a[:, None] // NH) == (a[None, :] // NH), 0.0, MASK_NEG).astype(f)

    def chunk2(v_):  # [C] -> [P, NCH]
        return np.stack([v_[m * P:(m + 1) * P] for m in range(NCH)], axis=1).astype(f)

    return {
        "wqk": wqk, "wv": wv_c, "pt": pt, "bqk": bqk,
        "bv": chunk2(bv), "pb": chunk2(proj_b),
        "nw": chunk2(norm_w), "nb": chunk2(norm_b),
        "ind": ind, "bc": bc, "mask": mask,
    }


def make_in_maps(x, norm_w, norm_b, qkv_w, qkv_b, proj_w, proj_b, prec=None):
    prec = PRECISION if prec is None else prec
    cd = np.float32 if prec == "f32r" else np.float16
    common = _host_inputs(x, norm_w, norm_b, qkv_w, qkv_b, proj_w, proj_b, prec)
    xr = np.ascontiguousarray(
        np.asarray(x, dtype=np.float32).reshape(B, C, N).astype(cd))
    in_maps = []
    for c in range(NCORES):
        m = dict(common)
        m["x"] = xr[c * NB:(c + 1) * NB]
        in_maps.append(m)
    return in_maps


def kernel(x, norm_w, norm_b, qkv_w, qkv_b, proj_w, proj_b):
    nc = _get_nc()
    in_maps = make_in_maps(x, norm_w, norm_b, qkv_w, qkv_b, proj_w, proj_b)
    res = run_bass_kernel_spmd(nc, in_maps, core_ids=list(range(NCORES)))
    y = np.concatenate([res.results[c]["y"] for c in range(NCORES)], axis=0)
    return y.reshape(B, C, H, W).astype(np.float32)


# revision 49
# speedup vs baseline: 1.0416x; 1.0251x over previous
"""AttentionBlock (GroupNorm + degenerate head-axis attention + proj + residual)
on 8 Trainium2 NeuronCores, data-parallel over batch (2 batches per core).

Reference math (B=16, C=256, H=W=64, NH=4, dh=64, N=HW=4096, G=8 groups):
  xn   = GroupNorm(8, C)(x) * norm_w + norm_b
  qkv  = qkv_w @ xn + qkv_b            (1x1 convs == channel GEMMs)
  q,k,v: [NH, dh, N]; attn[p,i,j] = softmax_j( (1/8) sum_n q[i,p,n] k[j,p,n] )
  out[(p,i), n] = sum_j attn[p,i,j] v[(p,j), n]
  y    = proj_w @ out + proj_b + x

Kernel strategy (per core):
  - channels on partitions (2 chunks of 128), spatial N on the free dim
  - GroupNorm folded into the qkv GEMM: W_eff = W * scale(c) on the
    contraction channel; the bias side (W @ shift + b) enters the attention
    Gram as exact rank-1 fp32 corrections
  - q/k are computed TRANSPOSED ([n, channel], channels permuted p-major:
    pm = p*4 + head) so the 64 per-position 4x4 Gram matrices land on the
    diagonal 4-blocks of two [128, 128] blocks, accumulated across 32
    spatial chunks in PSUM
  - softmax over 4-blocks via a mask-bias tile (off-block -> -1e30 -> exp 0)
  - attention out and proj are fused: y = (EN.T @ PT).T @ v so the softmaxed
    Gram never needs a transpose
  - PRECISION="f32r": float32r matmuls (~1e-4 matmul rel err, y err ~2e-4)
    PRECISION="f16":  float16 matmuls (y err ~1.5e-3), faster weight loads
"""
import numpy as np

import concourse.bacc as bacc
import concourse.mybir as mybir
import concourse.tile as tile
from concourse.bass_utils import run_bass_kernel_spmd

F32 = mybir.dt.float32
F32R = mybir.dt.float32r
F16 = mybir.dt.float16

NCORES = 8
B, C, H, W = 16, 256, 64, 64
N = H * W                    # 4096
NB = B // NCORES             # batches per core = 2
NH, DH, G = 4, 64, 8
EPS = 1e-5
P = 128
NCH = C // P                 # channel chunks = 2
NT128 = N // 128             # 32
NT512 = N // 512             # 8
MASK_NEG = -1.0e30

PRECISION = "f16"            # "f32r" | "f16"

# p-major channel permutation: pm index j = p*4 + h  <->  orig channel h*64 + p
_PM = np.arange(C)
ORIG_OF_PM = (_PM % NH) * DH + _PM // NH   # orig channel for p-major index

_DEBUG_ACCUM_Y = False


def _build(replicate=1, loop=1, prec=None):
    """loop>1 wraps the computation in a hardware For_i repeating it
    (identical result every iteration) — used only for wall-clock timing."""
    prec = PRECISION if prec is None else prec
    CD = F32R if prec == "f32r" else F16          # compute dtype for big GEMMs
    f32r_mode = CD is F32R

    nc = bacc.Bacc()
    x_d = nc.declare_dram_parameter("x", [NB, C, N], CD, isOutput=False)
    wqk_d = nc.declare_dram_parameter("wqk", [C, 512], F32R, isOutput=False)
    wqkh_d = nc.declare_dram_parameter("wqkh", [C, 512], CD, isOutput=False)
    wvr_d = nc.declare_dram_parameter("wvr", [C, C], CD, isOutput=False)
    ident_d = nc.declare_dram_parameter("ident", [P, P], CD, isOutput=False)
    wv_d = nc.declare_dram_parameter("wv", [C, C], F32R, isOutput=False)
    pt_d = nc.declare_dram_parameter("pt", [C, C], CD, isOutput=False)
    bqk_d = nc.declare_dram_parameter("bqk", [1, 512], F32, isOutput=False)
    bv_d = nc.declare_dram_parameter("bv", [P, NCH], F32, isOutput=False)
    pb_d = nc.declare_dram_parameter("pb", [P, NCH], F32, isOutput=False)
    nw_d = nc.declare_dram_parameter("nw", [P, NCH], F32, isOutput=False)
    nb_d = nc.declare_dram_parameter("nb", [P, NCH], F32, isOutput=False)
    ind_d = nc.declare_dram_parameter("ind", [P, NCH, G], F32, isOutput=False)
    bc_d = nc.declare_dram_parameter("bc", [G, NCH, P], F32, isOutput=False)
    mask_d = nc.declare_dram_parameter("mask", [P, P], F32, isOutput=False)
    YD = F32 if prec == "f32r" else F16
    y_d = nc.declare_dram_parameter("y", [NB, C, N], YD, isOutput=True)

    AOT = mybir.AluOpType
    AFT = mybir.ActivationFunctionType

    def f32view(ap):
        # DVE/ACT-readable view of a float32r AP
        return ap.bitcast(F32) if ap.dtype == F32R else ap

    with tile.TileContext(nc) as tc:
        with (
            tc.tile_pool(name="wpool", bufs=1) as wpool,       # weights/constants
            tc.tile_pool(name="xr", bufs=2) as xr_pool,        # x, per batch
            tc.tile_pool(name="qkt", bufs=6) as qkt_pool,
            tc.tile_pool(name="per_b", bufs=2) as pb_pool,     # per-batch smalls
            tc.tile_pool(name="ypool", bufs=5) as y_pool,
            tc.tile_pool(name="sm", bufs=3) as sm_pool,        # softmax temps
            tc.tile_pool(name="psb", bufs=3, space="PSUM") as ps_big,
            tc.tile_pool(name="psv", bufs=2, space="PSUM") as ps_vproj,
            tc.tile_pool(name="psg", bufs=2, space="PSUM") as ps_g,
            tc.tile_pool(name="pss", bufs=1, space="PSUM") as ps_small,
        ):
            # ---- load constants ----
            wqk_t = wpool.tile([P, NCH, 512], F32R)
            nc.sync.dma_start(wqk_t[:], wqk_d.rearrange("(m p) o -> p m o", p=P))
            wv_t = wpool.tile([P, NCH, C], F32R)
            nc.sync.dma_start(wv_t[:], wv_d.rearrange("(m p) o -> p m o", p=P))
            wqkh_t = wpool.tile([P, NCH, 512], CD)
            nc.sync.dma_start(wqkh_t[:], wqkh_d.rearrange("(m p) o -> p m o", p=P))
            wvr_t = wpool.tile([P, NCH, C], CD)
            nc.sync.dma_start(wvr_t[:], wvr_d.rearrange("(m p) o -> p m o", p=P))
            ident_t = wpool.tile([P, P], CD)
            nc.sync.dma_start(ident_t[:], ident_d[:])
            pt_t = wpool.tile([P, NCH, C], CD)
            nc.sync.dma_start(pt_t[:], pt_d.rearrange("(m p) o -> p m o", p=P))
            bqk_t = wpool.tile([1, 512], F32)
            nc.sync.dma_start(bqk_t[:], bqk_d[:])
            bv_t = wpool.tile([P, NCH], F32)
            nc.sync.dma_start(bv_t[:], bv_d[:])
            pb_t = wpool.tile([P, NCH], F32)
            nc.sync.dma_start(pb_t[:], pb_d[:])
            nw_t = wpool.tile([P, NCH], F32)
            nc.sync.dma_start(nw_t[:], nw_d[:])
            nb_t = wpool.tile([P, NCH], F32)
            nc.sync.dma_start(nb_t[:], nb_d[:])
            ind_t = wpool.tile([P, NCH, G], F32)
            nc.sync.dma_start(ind_t[:], ind_d[:])
            bc_t = wpool.tile([G, NCH, P], F32)
            nc.sync.dma_start(bc_t[:], bc_d[:])
            mask_t = wpool.tile([P, P], F32)
            nc.sync.dma_start(mask_t[:], mask_d[:])
            eps_t = wpool.tile([G, 1], F32)
            nc.vector.memset(eps_t[:], EPS)

            import contextlib
            loop_ctx = tc.For_i(0, loop, 1) if loop > 1 else contextlib.nullcontext()
            with loop_ctx:
              for b in [b for _ in range(replicate) for b in range(NB)]:
                # ================= load x + groupnorm stats =================
                x_r = xr_pool.tile([P, NCH, N], CD, tag="xr")
                stat2 = pb_pool.tile([P, NCH, 2], F32, tag="stat2")
                for m in range(NCH):
                    # half-granular DMA across both HWDGE rings
                    for q in range(2):
                        eng = nc.sync if (m + q) % 2 == 0 else nc.scalar
                        eng.dma_start(x_r[:, m, q * 2048:(q + 1) * 2048],
                                      x_d[b, m * P:(m + 1) * P, q * 2048:(q + 1) * 2048])
                    # per-channel mean/var over N
                    stats = pb_pool.tile([P, 8, 6], F32, tag="stats")
                    for j in range(8):
                        nc.vector.bn_stats(stats[:, j, :],
                                           f32view(x_r[:, m, j * 512:(j + 1) * 512]))
                    mv = pb_pool.tile([P, 2], F32, tag="mv")
                    nc.vector.bn_aggr(mv[:], stats[:])
                    # stat2 = (mean, var + mean^2)
                    nc.vector.tensor_copy(stat2[:, m, 0:1], mv[:, 0:1])
                    sq = pb_pool.tile([P, 1], F32, tag="sq")
                    nc.vector.tensor_mul(sq[:], mv[:, 0:1], mv[:, 0:1])
                    nc.vector.tensor_add(stat2[:, m, 1:2], mv[:, 1:2], sq[:])

                # group stats: S_g[g, :] = (mean_g, E[x^2]_g)   (IND holds 1/32)
                sg_ps = ps_small.tile([G, 2], F32, tag="small")
                for m in range(NCH):
                    nc.tensor.matmul(sg_ps[:], ind_t[:, m, :], stat2[:, m, :],
                                     start=(m == 0), stop=(m == NCH - 1))
                gs = pb_pool.tile([G, 2], F32, tag="gs")
                nc.vector.tensor_copy(gs[:], sg_ps[:])
                # var_g = E[x^2] - mean^2 ; rstd = 1/sqrt(var+eps)
                gsq = pb_pool.tile([G, 1], F32, tag="gsq")
                nc.vector.tensor_mul(gsq[:], gs[:, 0:1], gs[:, 0:1])
                gvar = pb_pool.tile([G, 1], F32, tag="gvar")
                nc.vector.tensor_tensor(gvar[:], gs[:, 1:2], gsq[:], AOT.subtract)
                grt = pb_pool.tile([G, 1], F32, tag="grt")
                nc.scalar.activation(grt[:], gvar[:], AFT.Sqrt, bias=eps_t[:])
                gstats = pb_pool.tile([G, 2], F32, tag="gstats")
                nc.vector.tensor_copy(gstats[:, 0:1], gs[:, 0:1])
                nc.vector.reciprocal(gstats[:, 1:2], grt[:])

                # per-channel scale/shift; scale folded into x (x_s = scale*x)
                x_s = pb_pool.tile([P, NCH, N], CD, tag="x_s")
                scale_keep = []
                # duplicated to width 2: fp32r matmuls need even free dims
                shiftw = pb_pool.tile([P, NCH, 2], F32R, tag="shiftw")
                sxs2 = pb_pool.tile([P, NCH, 2], F32R, tag="sxs2")
                for m in range(NCH):
                    bc_ps = ps_small.tile([P, 2], F32, tag="small")
                    nc.tensor.matmul(bc_ps[:], bc_t[:, m, :], gstats[:], start=True, stop=True)
                    scale_m = pb_pool.tile([P, 1], F32, tag="scale_m")
                    nc.vector.tensor_mul(scale_m[:], bc_ps[:, 1:2], nw_t[:, m:m + 1])
                    tmp_m = pb_pool.tile([P, 1], F32, tag="tmp_m")
                    nc.vector.tensor_mul(tmp_m[:], bc_ps[:, 0:1], scale_m[:])
                    # shift = norm_b - mean*scale
                    nc.vector.tensor_tensor(shiftw[:, m, 0:1],
                                            nb_t[:, m:m + 1], tmp_m[:], AOT.subtract)
                    nc.vector.tensor_tensor(shiftw[:, m, 1:2],
                                            nb_t[:, m:m + 1], tmp_m[:], AOT.subtract)
                    # sxs = scale * sum_n x = scale * N * mean
                    sxs_m = pb_pool.tile([P, 1], F32, tag="sxs_m")
                    nc.vector.tensor_mul(sxs_m[:], stat2[:, m, 0:1], scale_m[:])
                    nc.vector.tensor_scalar_mul(sxs2[:, m, 0:1], sxs_m[:], float(N))
                    nc.vector.tensor_scalar_mul(sxs2[:, m, 1:2], sxs_m[:], float(N))
                    # x_s = scale * x (per contraction channel); first halves
                    # first so the qkT sweep can start before the tail is scaled
                    nc.vector.tensor_scalar_mul(x_s[:, m, 0:N // 2],
                                                x_r[:, m, 0:N // 2], scale_m[:])
                    scale_keep.append(scale_m)

                for m in range(NCH):
                    nc.vector.tensor_scalar_mul(x_s[:, m, N // 2:N],
                                                x_r[:, m, N // 2:N], scale_keep[m][:])

                # qk effective bias row: beff = W.T @ shift + bqk   [1, 512]
                bq_ps = ps_small.tile([2, 512], F32, tag="small")
                for m in range(NCH):
                    nc.tensor.matmul(bq_ps[:], shiftw[:, m, :], wqk_t[:, m, :],
                                     start=(m == 0), stop=(m == NCH - 1))
                bq_sb = pb_pool.tile([1, 512], F32, tag="bq_sb")
                nc.vector.tensor_tensor(bq_sb[:], bq_ps[0:1, :], bqk_t[:], AOT.add)

                # wsx = W_eff.T @ sum_n x = W.T @ (scale * sx)  -> [1, 512]
                wsx_ps = ps_small.tile([2, 512], F32, tag="small")
                for m in range(NCH):
                    nc.tensor.matmul(wsx_ps[:], sxs2[:, m, :], wqk_t[:, m, :],
                                     start=(m == 0), stop=(m == NCH - 1))
                wsx_sb = pb_pool.tile([1, 512], F32, tag="wsx_sb")
                nc.vector.tensor_copy(wsx_sb[:], wsx_ps[0:1, :])
                # wcomb = Wk_eff sx + N * beff_k
                nbk = pb_pool.tile([1, C], F32, tag="nbk")
                nc.vector.tensor_scalar_mul(nbk[:], bq_sb[0:1, C:2 * C], float(N))
                wcomb = pb_pool.tile([1, C], F32, tag="wcomb")
                nc.vector.tensor_tensor(wcomb[:], wsx_sb[0:1, C:2 * C], nbk[:], AOT.add)

                # v bias: bveff[:, oc] = Wv.T @ shift + bv
                bveff = pb_pool.tile([P, NCH], F32, tag="bveff")
                bveff2 = pb_pool.tile([P, NCH, 2], CD, tag="bveff2")
                for oc in range(NCH):
                    bv_ps = ps_small.tile([P, 2], F32, tag="small")
                    for m in range(NCH):
                        nc.tensor.matmul(bv_ps[:], wv_t[:, m, oc * P:(oc + 1) * P],
                                         shiftw[:, m, :],
                                         start=(m == 0), stop=(m == NCH - 1))
                    nc.vector.tensor_tensor(bveff[:, oc:oc + 1], bv_ps[:, 0:1],
                                            bv_t[:, oc:oc + 1], AOT.add)
                    nc.vector.tensor_copy(bveff2[:, oc, 0:1], bveff[:, oc:oc + 1])
                    nc.vector.tensor_copy(bveff2[:, oc, 1:2], bveff[:, oc:oc + 1])

                # ================= qkT sweep + Gram accumulation =================
                # qkT tiles hold UNBIASED q0/k0 (channels p-major, q pre-scaled
                # by 1/8 on the host); biases enter as rank-1 fp32 terms:
                #   G = q0 k0' + beffq (Wk_eff sx + N beffk)' + (Wq_eff sx) beffk'
                g_ps = [ps_g.tile([P, C], F32, tag="g", name=f"g_ps_{b}_{m}") for m in range(NCH)]
                for t in range(NT128):
                    qk_ps = ps_big.tile([P, 512], F32, tag="big")
                    for m in range(NCH):
                        nc.tensor.matmul(qk_ps[:], x_s[:, m, t * P:(t + 1) * P],
                                         wqkh_t[:, m, :],
                                         start=(m == 0), stop=(m == NCH - 1))
                    qkt = qkt_pool.tile([P, 512], CD, tag="qkt")
                    if t % 2 == 0:
                        nc.vector.tensor_copy(qkt[:], qk_ps[:])
                    else:
                        nc.scalar.copy(qkt[:], qk_ps[:])
                    if f32r_mode:
                        # f32r needs moving free >= 256: compute full k columns
                        for m in range(NCH):
                            nc.tensor.matmul(g_ps[m][:], qkt[:, m * P:(m + 1) * P],
                                             qkt[:, 256:512],
                                             start=(t == 0), stop=False)
                    else:
                        # f16 full-rate at N=128: only the relevant k chunk
                        for m in range(NCH):
                            nc.tensor.matmul(g_ps[m][:, m * P:(m + 1) * P],
                                             qkt[:, m * P:(m + 1) * P],
                                             qkt[:, 256 + m * P:256 + (m + 1) * P],
                                             start=(t == 0), stop=False)
                # rank-1 bias corrections (plain fp32 — exact)
                for m in range(NCH):
                    if f32r_mode:
                        nc.tensor.matmul(g_ps[m][:], bq_sb[0:1, m * P:(m + 1) * P],
                                         wcomb[:], start=False, stop=False)
                        nc.tensor.matmul(g_ps[m][:], wsx_sb[0:1, m * P:(m + 1) * P],
                                         bq_sb[0:1, C:2 * C], start=False, stop=True)
                    else:
                        nc.tensor.matmul(g_ps[m][:, m * P:(m + 1) * P],
                                         bq_sb[0:1, m * P:(m + 1) * P],
                                         wcomb[0:1, m * P:(m + 1) * P],
                                         start=False, stop=False)
                        nc.tensor.matmul(g_ps[m][:, m * P:(m + 1) * P],
                                         wsx_sb[0:1, m * P:(m + 1) * P],
                                         bq_sb[0:1, C + m * P:C + (m + 1) * P],
                                         start=False, stop=True)

                # ================= softmax + QT =================
                qt_t = pb_pool.tile([P, NCH, C], CD, tag="qt")
                for m in range(NCH):
                    grel = g_ps[m][:, m * P:(m + 1) * P]
                    s_t = sm_pool.tile([P, P], F32, tag="s")
                    nc.vector.tensor_tensor(s_t[:], grel, mask_t[:], AOT.add)
                    mx = sm_pool.tile([P, 1], F32, tag="mx")
                    nc.vector.reduce_max(mx[:], s_t[:], axis=mybir.AxisListType.X)
                    mxn = sm_pool.tile([P, 1], F32, tag="mxn")
                    nc.vector.tensor_scalar_mul(mxn[:], mx[:], -1.0)
                    e_t = sm_pool.tile([P, P], F32, tag="e")
                    esum = sm_pool.tile([P, 1], F32, tag="esum")
                    nc.scalar.activation(e_t[:], s_t[:], AFT.Exp, bias=mxn[:], accum_out=esum[:])
                    erec = sm_pool.tile([P, 1], F32, tag="erec")
                    nc.vector.reciprocal(erec[:], esum[:])
                    en_t = sm_pool.tile([P, P], CD, tag="en")
                    nc.vector.tensor_scalar_mul(en_t[:], e_t[:], erec[:])
                    # QT_m = EN_m.T @ PT_m   (softmaxed Gram enters transposed)
                    qt_ps = ps_small.tile([P, C], F32, tag="small")
                    nc.tensor.matmul(qt_ps[:], en_t[:], pt_t[:, m, :], start=True, stop=True)
                    nc.vector.tensor_copy(qt_t[:, m, :], qt_ps[:])

                # fused attention*value*proj matrix: M2T[c, o] = sum_pj Wv[pj, c] QT[pj, o]
                # (the v GEMM never materializes: y_main = M2T.T @ x_s)
                m2t = pb_pool.tile([P, NCH, C], CD, tag="m2t")
                for cc in range(NCH):
                    m2_ps = ps_small.tile([P, C], F32, tag="small")
                    for pjc in range(NCH):
                        nc.tensor.matmul(m2_ps[:], wvr_t[:, pjc, cc * P:(cc + 1) * P],
                                         qt_t[:, pjc, :],
                                         start=(pjc == 0), stop=(pjc == NCH - 1))
                    nc.vector.tensor_copy(m2t[:, cc, :], m2_ps[:])
                # pbeff = proj_b + QT.T @ bveff  (v bias folded per output channel)
                pbeff = pb_pool.tile([P, NCH], F32, tag="pbeff")
                for oc in range(NCH):
                    pbe_ps = ps_small.tile([P, 2], F32, tag="small")
                    for pjc in range(NCH):
                        nc.tensor.matmul(pbe_ps[:], qt_t[:, pjc, oc * P:(oc + 1) * P],
                                         bveff2[:, pjc, :],
                                         start=(pjc == 0), stop=(pjc == NCH - 1))
                    nc.vector.tensor_tensor(pbeff[:, oc:oc + 1], pbe_ps[:, 0:1],
                                            pb_t[:, oc:oc + 1], AOT.add)

                # ================= y = M2T.T @ x_s + x + pbeff =================
                for nt in range(NT512):
                    for oc in range(NCH):
                        y_ps = ps_vproj.tile([P, 512], F32, tag="vproj", name="y_ps")
                        for m in range(NCH):
                            nc.tensor.matmul(y_ps[:], m2t[:, m, oc * P:(oc + 1) * P],
                                             x_s[:, m, nt * 512:(nt + 1) * 512],
                                             start=(m == 0), stop=False)
                        # residual folded into PSUM: += I.T @ x
                        nc.tensor.matmul(y_ps[:], ident_t[:],
                                         x_r[:, oc, nt * 512:(nt + 1) * 512],
                                         start=False, stop=True)
                        y_sb = y_pool.tile([P, 512], YD, tag="y", name="y_sb")
                        if (nt * NCH + oc) % 2 == 0:
                            nc.vector.tensor_scalar_add(y_sb[:], y_ps[:],
                                                        pbeff[:, oc:oc + 1])
                        else:
                            nc.scalar.activation(y_sb[:], y_ps[:], AFT.Identity,
                                                 bias=pbeff[:, oc:oc + 1])
                        if _DEBUG_ACCUM_Y:
                            nc.gpsimd.dma_start(
                                y_d[b, oc * P:(oc + 1) * P, nt * 512:(nt + 1) * 512],
                                y_sb[:], accum_op=AOT.add)
                        else:
                            seng = nc.sync if (nt * NCH + oc) % 5 == 0 else nc.scalar
                            seng.dma_start(
                                y_d[b, oc * P:(oc + 1) * P, nt * 512:(nt + 1) * 512],
                                y_sb[:])

    if not nc.is_finalized():
        nc.finalize()
    return nc


_NC_CACHE = {}


def _get_nc(replicate=1, loop=1, prec=None):
    prec = PRECISION if prec is None else prec
    key = (replicate, loop, prec)
    if key not in _NC_CACHE:
        _NC_CACHE[key] = _build(replicate, loop, prec)
    return _NC_CACHE[key]


def _host_inputs(x, norm_w, norm_b, qkv_w, qkv_b, proj_w, proj_b, prec):
    """Host-side weight preprocessing -> per-core common input dict."""
    f = np.float32
    cd = np.float32 if prec == "f32r" else np.float16
    norm_w, norm_b = np.asarray(norm_w, f), np.asarray(norm_b, f)
    qkv_w, qkv_b = np.asarray(qkv_w, f), np.asarray(qkv_b, f)
    proj_w, proj_b = np.asarray(proj_w, f), np.asarray(proj_b, f)

    perm = ORIG_OF_PM
    wq = qkv_w[0:C][perm] / 8.0          # fold attention scale dh^-0.5 = 1/8
    wk = qkv_w[C:2 * C][perm]
    wv = qkv_w[2 * C:3 * C][perm]
    bq = qkv_b[0:C][perm] / 8.0
    bk = qkv_b[C:2 * C][perm]
    bv = qkv_b[2 * C:3 * C][perm]

    wqk = np.concatenate([wq.T, wk.T], axis=1).astype(f)      # [C, 512]
    bqk = np.concatenate([bq, bk])[None, :].astype(f)         # [1, 512]
    wv_c = np.ascontiguousarray(wv.T).astype(f)               # [C, C] (c_in, o_pm)
    pt = np.ascontiguousarray(proj_w[:, perm].T).astype(cd)   # [C(pm), C(orig o)]

    ch = np.arange(C)
    ind = np.zeros((P, NCH, G), f)
    bc = np.zeros((G, NCH, P), f)
    for m in range(NCH):
        grp = (ch[m * P:(m + 1) * P]) // (C // G)
        for c0 in range(P):
            ind[c0, m, grp[c0]] = 1.0 / (C // G)
            bc[grp[c0], m, c0] = 1.0
    a = np.arange(P)
    mask = np.where((a[:, None] // NH) == (a[None, :] // NH), 0.0, MASK_NEG).astype(f)

    def chunk2(v_):  # [C] -> [P, NCH]
        return np.stack([v_[m * P:(m + 1) * P] for m in range(NCH)], axis=1).astype(f)

    return {
        "wqk": wqk, "wqkh": wqk.astype(cd), "wv": wv_c,
        "wvr": np.ascontiguousarray(wv).astype(cd),
        "pt": pt, "bqk": bqk,
        "bv": chunk2(bv), "pb": chunk2(proj_b),
        "nw": chunk2(norm_w), "nb": chunk2(norm_b),
        "ind": ind, "bc": bc, "mask": mask,
        "ident": np.eye(P, dtype=cd),
    }


def make_in_maps(x, norm_w, norm_b, qkv_w, qkv_b, proj_w, proj_b, prec=None):
    prec = PRECISION if prec is None else prec
    cd = np.float32 if prec == "f32r" else np.float16
    common = _host_inputs(x, norm_w, norm_b, qkv_w, qkv_b, proj_w, proj_b, prec)
    xr = np.ascontiguousarray(
        np.asarray(x, dtype=np.float32).reshape(B, C, N).astype(cd))
    in_maps = []
    for c in range(NCORES):
        m = dict(common)
        m["x"] = xr[c * NB:(c + 1) * NB]
        in_maps.append(m)
    return in_maps


def kernel(x, norm_w, norm_b, qkv_w, qkv_b, proj_w, proj_b):
    nc = _get_nc()
    in_maps = make_in_maps(x, norm_w, norm_b, qkv_w, qkv_b, proj_w, proj_b)
    res = run_bass_kernel_spmd(nc, in_maps, core_ids=list(range(NCORES)))
    y = np.concatenate([res.results[c]["y"] for c in range(NCORES)], axis=0)
    return y.reshape(B, C, H, W).astype(np.float32)
